# revision 10
# baseline (speedup 1.0000x reference)
"""Trainium2 Bass kernel for HalfHadamardTrustQuantizer.

Computation (forward value of the reference, which collapses to xq):
  x_had = blockwise-64 Hadamard rotation of channels:  (B,C,H,W), C=512 = 8 groups of 64
  std   = sqrt(mean(x_had^2)) per sample  (== RMS of x by orthogonality)
  scale = OPT*std + 1e-8 ; step = 2*scale/15
  xq    = round(clip(x_had,-scale,scale)/step + 0.5)*step - step/2

Sharding: data-parallel over batch; 2 samples per core on 8 cores.

Per-core pipeline (2 samples of (512, 3136) fp32):
  phase A: DMA block-rows (128,3136) in; ACT Square+accum_out -> per-partition sumsq
  scalars: PE ones-matmul cross-partition reduce+broadcast; ACT Sqrt + one DVE
           Newton step; reciprocal for 1/step
  phase B: PE fp32 matmul with blockdiag(aux,aux) weights (N=512 chunks);
           ACT Identity(scale=1/step, bias=0.5) PSUM -> int32 (RNE round fused);
           DVE int clip (min 8, max -7); DVE affine int->f32 (*step - step/2);
           DMA out
"""

import numpy as np
from contextlib import ExitStack

B, C, HH, WW = 16, 512, 56, 56
R = HH * WW            # 3136 spatial
NCORES = 8
S = B // NCORES        # samples per core
NB = C // 128          # block-rows per sample
N_ELEM = C * R         # per-sample reduction size
OPT = 2.513930578568423
INV_N = np.float64(1.0) / np.float64(N_ELEM)   # cast at use
TWO_15 = np.float32(2.0) / np.float32(15.0)

_CACHE = {}


def _build_program():
    import concourse.bacc as bacc
    import concourse.tile as tile
    import concourse.mybir as mybir

    AF = mybir.ActivationFunctionType
    OP = mybir.AluOpType
    f32 = mybir.dt.float32
    i32 = mybir.dt.int32

    nc = bacc.Bacc("TRN2", target_bir_lowering=False, debug=False,
                   num_devices=NCORES)
    x = nc.dram_tensor("x", [S * C, R], f32, kind="ExternalInput").ap()
    w = nc.dram_tensor("w", [128, 128], f32, kind="ExternalInput").ap()
    y = nc.dram_tensor("y", [S * C, R], f32, kind="ExternalOutput").ap()

    # matmul chunks grouped in pairs into 2-bank PSUM tiles; ACT drains a
    # whole tile at once
    TILES = [(0, [512, 512]), (1024, [512, 512]), (2048, [512, 512]),
             (3072, [64])]
    # output halves for clip/affine/store, aligned to drain tiles
    HALVES = [(0, 2048), (2048, 1088)]

    with tile.TileContext(nc) as tc, ExitStack() as ctx:
        xp = ctx.enter_context(tc.tile_pool(name="xp", bufs=8))
        cn = ctx.enter_context(tc.tile_pool(name="cn", bufs=1))
        sq = ctx.enter_context(tc.tile_pool(name="sq", bufs=1))
        ac = ctx.enter_context(tc.tile_pool(name="ac", bufs=2))
        sc = ctx.enter_context(tc.tile_pool(name="sc", bufs=2))
        ip = ctx.enter_context(tc.tile_pool(name="ip", bufs=2))
        op_ = ctx.enter_context(tc.tile_pool(name="op", bufs=3))
        pp = ctx.enter_context(tc.tile_pool(name="pp", bufs=4, space="PSUM"))

        wt = cn.tile([128, 128], f32, tag="w")
        nc.sync.dma_start(wt[:], w[:])
        ones = cn.tile([128, 128], f32, tag="ones")
        nc.gpsimd.memset(ones[:], 1.0)
        half = cn.tile([128, 1], f32, tag="half")
        nc.gpsimd.memset(half[:], 0.5)

        sq_scr = sq.tile([128, R], f32, tag="sqscr")

        xts = {}
        scal = {}

        def phase_a_row(s, b):
            # load one block-row (split across two DMA queues) and
            # accumulate its sum of squares (DVE for s0, ACT for s1 --
            # keeps either engine from becoming the single hot spot)
            xt = xp.tile([128, R], f32, tag="xrow")
            r0 = x[s * C + b * 128:s * C + (b + 1) * 128, :]
            nc.sync.dma_start(xt[:, 0:1568], r0[:, 0:1568])
            nc.scalar.dma_start(xt[:, 1568:R], r0[:, 1568:R])
            xts[(s, b)] = xt
            if s == 0:
                nc.vector.scalar_tensor_tensor(sq_scr[:], xt[:], 1.0, xt[:],
                                               OP.mult, OP.mult,
                                               accum_out=parts[s][:, b:b + 1])
            else:
                nc.scalar.activation(sq_scr[:], xt[:], AF.Square,
                                     accum_out=parts[s][:, b:b + 1])

        def sample_scalars(s):
            # ---- per-sample scalars ----
            part = parts[s]
            red = sc.tile([128, 1], f32, tag="red")
            nc.vector.reduce_sum(red[:], part[:], axis=mybir.AxisListType.X)
            tot = pp.tile([128, 1024], f32, tag="pchunk")
            tot = tot[:, 0:1]
            nc.tensor.matmul(tot[:], ones[:], red[:], start=True, stop=True)
            std0 = sc.tile([128, 1], f32, tag="std0")
            nc.scalar.activation(std0[:], tot[:], AF.Sqrt, scale=float(INV_N))
            # one Newton step: std1 = 0.5*std0 + (0.5/N)*(tot * (1/(std0+eps)))
            std0p = sc.tile([128, 1], f32, tag="std0p")
            nc.vector.tensor_scalar_add(std0p[:], std0[:], 1e-30)
            rstd = sc.tile([128, 1], f32, tag="rstd")
            nc.vector.reciprocal(rstd[:], std0p[:])
            t1 = sc.tile([128, 1], f32, tag="t1")
            nc.vector.tensor_tensor(t1[:], tot[:], rstd[:], OP.mult)
            hs = sc.tile([128, 1], f32, tag="hs")
            nc.vector.tensor_scalar_mul(hs[:], std0[:], 0.5)
            std1 = sc.tile([128, 1], f32, tag="std1")
            nc.vector.scalar_tensor_tensor(std1[:], t1[:], float(0.5 * INV_N),
                                           hs[:], OP.mult, OP.add)
            scale_t = sc.tile([128, 1], f32, tag="scale")
            nc.vector.tensor_scalar(scale_t[:], std1[:], float(OPT), 1e-8,
                                    OP.mult, OP.add)
            step = sc.tile([128, 1], f32, tag="step")
            nc.vector.tensor_scalar_mul(step[:], scale_t[:], float(TWO_15))
            inv = sc.tile([128, 1], f32, tag="inv")
            nc.vector.reciprocal(inv[:], step[:])
            hstep = sc.tile([128, 1], f32, tag="hstep")
            nc.vector.tensor_scalar_mul(hstep[:], step[:], 0.5)
            scal[s] = (inv, step, hstep)

        prefilled = {}

        def emit_tile_mm(s, b, off, chunks):
            xt = xts[(s, b)]
            pm = pp.tile([128, 1024], f32, tag="pchunk")
            co = 0
            for ch in chunks:
                nc.tensor.matmul(pm[:, co:co + ch], wt[:],
                                 xt[:, off + co:off + co + ch],
                                 start=True, stop=True)
                co += ch
            prefilled[(s, b, off)] = pm
            return pm

        def phase_b_row(s, b):
            inv, step, hstep = scal[s]
            # ---- phase B: rotate + quantize + store ----
            if True:
                irow = ip.tile([128, R], i32, tag="irow")
                for off, chunks in TILES:
                    tw = sum(chunks)
                    pm = prefilled.pop((s, b, off), None)
                    if pm is None:
                        pm = emit_tile_mm(s, b, off, chunks)
                    nc.scalar.activation(irow[:, off:off + tw],
                                         pm[:, :tw], AF.Identity,
                                         bias=half[:], scale=inv[:])
                orow = op_.tile([128, R], f32, tag="orow")
                for off, w_ in HALVES:
                    nc.gpsimd.tensor_scalar(irow[:, off:off + w_],
                                            irow[:, off:off + w_], 8, -7,
                                            OP.min, OP.max)
                    nc.vector.tensor_scalar(orow[:, off:off + w_],
                                            irow[:, off:off + w_],
                                            step[:], hstep[:],
                                            OP.mult, OP.subtract)
                    eng = nc.gpsimd if off == 0 else nc.sync
                    eng.dma_start(
                        y[s * C + b * 128:s * C + (b + 1) * 128,
                          off:off + w_], orow[:, off:off + w_])

        # ---- pipelined emission order ----
        # s0 loads, s0 scalars, then s0 phase-B rows interleaved with s1
        # loads (avoids FIFO head-of-line blocking on every engine), then
        # s1 scalars and s1 phase B.
        parts = {}
        for s in range(S):
            part_t = ac.tile([128, NB], f32, tag=f"part{s}", name=f"part{s}")
            parts[s] = part_t
        for b in range(NB):
            phase_a_row(0, b)
        for off, chunks in TILES[:3]:
            emit_tile_mm(0, 0, off, chunks)
        sample_scalars(0)
        for b in range(NB):
            phase_a_row(1, b)
            phase_b_row(0, b)
        sample_scalars(1)
        for b in range(NB):
            phase_b_row(1, b)
    nc.compile()
    return nc


def _get_program():
    if "nc" not in _CACHE:
        _CACHE["nc"] = _build_program()
    return _CACHE["nc"]


def kernel(x: np.ndarray, aux_matrix: np.ndarray) -> np.ndarray:
    from concourse.bass_utils import run_bass_kernel_spmd

    x = np.ascontiguousarray(x, dtype=np.float32)
    aux = np.ascontiguousarray(aux_matrix, dtype=np.float32)
    w128 = np.zeros((128, 128), dtype=np.float32)
    w128[:64, :64] = aux
    w128[64:, 64:] = aux

    nc = _get_program()
    in_maps = [
        {"x": x[c * S:(c + 1) * S].reshape(S * C, R), "w": w128}
        for c in range(NCORES)
    ]
    res = run_bass_kernel_spmd(nc, in_maps, list(range(NCORES)))
    out = np.empty((B, C, HH, WW), dtype=np.float32)
    for c in range(NCORES):
        out[c * S:(c + 1) * S] = res.results[c]["y"].reshape(S, C, HH, WW)
    return out


# revision 11
# speedup vs baseline: 1.0442x; 1.0442x over previous
"""Trainium2 Bass kernel for HalfHadamardTrustQuantizer.

Computation (forward value of the reference, which collapses to xq):
  x_had = blockwise-64 Hadamard rotation of channels:  (B,C,H,W), C=512 = 8 groups of 64
  std   = sqrt(mean(x_had^2)) per sample  (== RMS of x by orthogonality)
  scale = OPT*std + 1e-8 ; step = 2*scale/15
  xq    = round(clip(x_had,-scale,scale)/step + 0.5)*step - step/2

Sharding: data-parallel over batch; 2 samples per core on 8 cores.

Per-core pipeline (2 samples of (512, 3136) fp32):
  phase A: DMA block-rows (128,3136) in; ACT Square+accum_out -> per-partition sumsq
  scalars: PE ones-matmul cross-partition reduce+broadcast; ACT Sqrt + one DVE
           Newton step; reciprocal for 1/step
  phase B: PE fp32 matmul with blockdiag(aux,aux) weights (N=512 chunks);
           ACT Identity(scale=1/step, bias=0.5) PSUM -> int32 (RNE round fused);
           DVE int clip (min 8, max -7); DVE affine int->f32 (*step - step/2);
           DMA out
"""

import numpy as np
from contextlib import ExitStack

B, C, HH, WW = 16, 512, 56, 56
R = HH * WW            # 3136 spatial
NCORES = 8
S = B // NCORES        # samples per core
NB = C // 128          # block-rows per sample
N_ELEM = C * R         # per-sample reduction size
OPT = 2.513930578568423
INV_N = np.float64(1.0) / np.float64(N_ELEM)   # cast at use
TWO_15 = np.float32(2.0) / np.float32(15.0)

_CACHE = {}


def _build_program():
    import concourse.bacc as bacc
    import concourse.tile as tile
    import concourse.mybir as mybir

    AF = mybir.ActivationFunctionType
    OP = mybir.AluOpType
    f32 = mybir.dt.float32
    i32 = mybir.dt.int32

    nc = bacc.Bacc("TRN2", target_bir_lowering=False, debug=False,
                   num_devices=NCORES)
    x = nc.dram_tensor("x", [S * C, R], f32, kind="ExternalInput").ap()
    w = nc.dram_tensor("w", [128, 128], f32, kind="ExternalInput").ap()
    y = nc.dram_tensor("y", [S * C, R], f32, kind="ExternalOutput").ap()

    # matmul chunks grouped in pairs into 2-bank PSUM tiles; ACT drains a
    # whole tile at once
    TILES = [(0, [512, 512]), (1024, [512, 512]), (2048, [512, 512]),
             (3072, [64])]
    # output halves for clip/affine/store, aligned to drain tiles
    HALVES = [(0, 2048), (2048, 1088)]

    with tile.TileContext(nc) as tc, ExitStack() as ctx:
        xp = ctx.enter_context(tc.tile_pool(name="xp", bufs=8))
        cn = ctx.enter_context(tc.tile_pool(name="cn", bufs=1))
        sq = ctx.enter_context(tc.tile_pool(name="sq", bufs=1))
        ac = ctx.enter_context(tc.tile_pool(name="ac", bufs=2))
        sc = ctx.enter_context(tc.tile_pool(name="sc", bufs=2))
        ip = ctx.enter_context(tc.tile_pool(name="ip", bufs=2))
        op_ = ctx.enter_context(tc.tile_pool(name="op", bufs=3))
        pp = ctx.enter_context(tc.tile_pool(name="pp", bufs=4, space="PSUM"))

        wt = cn.tile([128, 128], f32, tag="w")
        nc.sync.dma_start(wt[:], w[:])
        ones = cn.tile([128, 128], f32, tag="ones")
        nc.gpsimd.memset(ones[:], 1.0)
        half = cn.tile([128, 1], f32, tag="half")
        nc.gpsimd.memset(half[:], 0.5)

        sq_scr = sq.tile([128, R], f32, tag="sqscr")

        xts = {}
        scal = {}

        def phase_a_row(s, b):
            # load one block-row (split across two DMA queues) and
            # accumulate its sum of squares (DVE for s0, ACT for s1 --
            # keeps either engine from becoming the single hot spot)
            xt = xp.tile([128, R], f32, tag="xrow")
            r0 = x[s * C + b * 128:s * C + (b + 1) * 128, :]
            nc.sync.dma_start(xt[:, 0:1568], r0[:, 0:1568])
            nc.scalar.dma_start(xt[:, 1568:R], r0[:, 1568:R])
            xts[(s, b)] = xt
            if s == 0:
                nc.vector.scalar_tensor_tensor(sq_scr[:], xt[:], 1.0, xt[:],
                                               OP.mult, OP.mult,
                                               accum_out=parts[s][:, b:b + 1])
            else:
                nc.scalar.activation(sq_scr[:], xt[:], AF.Square,
                                     accum_out=parts[s][:, b:b + 1])

        def sample_scalars(s):
            # ---- per-sample scalars ----
            part = parts[s]
            red = sc.tile([128, 1], f32, tag="red")
            nc.vector.reduce_sum(red[:], part[:], axis=mybir.AxisListType.X)
            tot = pp.tile([128, 1024], f32, tag="pchunk")
            tot = tot[:, 0:1]
            nc.tensor.matmul(tot[:], ones[:], red[:], start=True, stop=True)
            std0 = sc.tile([128, 1], f32, tag="std0")
            nc.scalar.activation(std0[:], tot[:], AF.Sqrt, scale=float(INV_N))
            # one Newton step: std1 = 0.5*std0 + (0.5/N)*(tot * (1/(std0+eps)))
            std0p = sc.tile([128, 1], f32, tag="std0p")
            nc.vector.tensor_scalar_add(std0p[:], std0[:], 1e-30)
            rstd = sc.tile([128, 1], f32, tag="rstd")
            nc.vector.reciprocal(rstd[:], std0p[:])
            t1 = sc.tile([128, 1], f32, tag="t1")
            nc.vector.tensor_tensor(t1[:], tot[:], rstd[:], OP.mult)
            hs = sc.tile([128, 1], f32, tag="hs")
            nc.vector.tensor_scalar_mul(hs[:], std0[:], 0.5)
            std1 = sc.tile([128, 1], f32, tag="std1")
            nc.vector.scalar_tensor_tensor(std1[:], t1[:], float(0.5 * INV_N),
                                           hs[:], OP.mult, OP.add)
            scale_t = sc.tile([128, 1], f32, tag="scale")
            nc.vector.tensor_scalar(scale_t[:], std1[:], float(OPT), 1e-8,
                                    OP.mult, OP.add)
            step = sc.tile([128, 1], f32, tag="step")
            nc.vector.tensor_scalar_mul(step[:], scale_t[:], float(TWO_15))
            inv = sc.tile([128, 1], f32, tag="inv")
            nc.vector.reciprocal(inv[:], step[:])
            hstep = sc.tile([128, 1], f32, tag="hstep")
            nc.vector.tensor_scalar_mul(hstep[:], step[:], 0.5)
            scal[s] = (inv, step, hstep)

        prefilled = {}

        def emit_tile_mm(s, b, off, chunks):
            xt = xts[(s, b)]
            pm = pp.tile([128, 1024], f32, tag="pchunk")
            co = 0
            for ch in chunks:
                nc.tensor.matmul(pm[:, co:co + ch], wt[:],
                                 xt[:, off + co:off + co + ch],
                                 start=True, stop=True)
                co += ch
            prefilled[(s, b, off)] = pm
            return pm

        def phase_b_row(s, b):
            inv, step, hstep = scal[s]
            # ---- phase B: rotate + quantize + store ----
            if True:
                irow = ip.tile([128, R], i32, tag="irow")
                for off, chunks in TILES:
                    tw = sum(chunks)
                    pm = prefilled.pop((s, b, off), None)
                    if pm is None:
                        pm = emit_tile_mm(s, b, off, chunks)
                    nc.scalar.activation(irow[:, off:off + tw],
                                         pm[:, :tw], AF.Identity,
                                         bias=half[:], scale=inv[:])
                orow = op_.tile([128, R], f32, tag="orow")
                for off, w_ in HALVES:
                    nc.gpsimd.tensor_scalar(irow[:, off:off + w_],
                                            irow[:, off:off + w_], 8, -7,
                                            OP.min, OP.max)
                    nc.vector.tensor_scalar(orow[:, off:off + w_],
                                            irow[:, off:off + w_],
                                            step[:], hstep[:],
                                            OP.mult, OP.subtract)
                    eng = nc.gpsimd if off == 0 else nc.sync
                    eng.dma_start(
                        y[s * C + b * 128:s * C + (b + 1) * 128,
                          off:off + w_], orow[:, off:off + w_])

        # ---- pipelined emission order ----
        # s0 loads, s0 scalars, then s0 phase-B rows interleaved with s1
        # loads (avoids FIFO head-of-line blocking on every engine), then
        # s1 scalars and s1 phase B.
        parts = {}
        for s in range(S):
            part_t = ac.tile([128, NB], f32, tag=f"part{s}", name=f"part{s}")
            parts[s] = part_t
        for b in range(NB):
            phase_a_row(0, b)
        sample_scalars(0)
        for b in range(NB):
            phase_a_row(1, b)
            phase_b_row(0, b)
        sample_scalars(1)
        for b in range(NB):
            phase_b_row(1, b)
    nc.compile()
    return nc


def _get_program():
    if "nc" not in _CACHE:
        _CACHE["nc"] = _build_program()
    return _CACHE["nc"]


def kernel(x: np.ndarray, aux_matrix: np.ndarray) -> np.ndarray:
    from concourse.bass_utils import run_bass_kernel_spmd

    x = np.ascontiguousarray(x, dtype=np.float32)
    aux = np.ascontiguousarray(aux_matrix, dtype=np.float32)
    w128 = np.zeros((128, 128), dtype=np.float32)
    w128[:64, :64] = aux
    w128[64:, 64:] = aux

    nc = _get_program()
    in_maps = [
        {"x": x[c * S:(c + 1) * S].reshape(S * C, R), "w": w128}
        for c in range(NCORES)
    ]
    res = run_bass_kernel_spmd(nc, in_maps, list(range(NCORES)))
    out = np.empty((B, C, HH, WW), dtype=np.float32)
    for c in range(NCORES):
        out[c * S:(c + 1) * S] = res.results[c]["y"].reshape(S, C, HH, WW)
    return out
